# revision 3
# baseline (speedup 1.0000x reference)
"""Depthwise causal conv1d kernel for Trainium2 (8 NeuronCores, SPMD).

Problem: x [B=8, T=4096, C=512] f32, weight [C=512, K=4] f32.
out[b, t, c] = sum_k weight[c, k] * x[b, t - 3 + k, c]   (causal, zero-pad)

Strategy (v2):
  - Data-parallel over batch: core b handles x[b].
  - Host-side layout: channels-first x[b].T padded with 3 leading zeros
    along time -> [C=512, T+3=4099] in fp16, reshaped to [128, 4*4099]
    (4 channel chunks of 128 on partitions). fp16 halves HBM traffic;
    accumulation stays fp32 in PSUM.
  - PE computes taps 0..2 as accumulating diag-matmuls (96 x 512-col
    matmuls ~= 23us); DVE fuses tap 3 + PSUM drain + fp16 cast in ONE
    scalar_tensor_tensor per half-chunk: out = (x_shift3 * w3) + psum.
    This cuts PE time 25% vs the 4-tap baseline and replaces the
    separate PSUM->SBUF copy.
  - Diag stationaries are built on the Scalar engine (activation
    scaled-copy of a host-sent fp16 identity), replacing the slow
    GpSimd affine_select path that delayed PE start by ~4us.
  - DMA: each dma_start serializes its descriptors on one of 16 rings
    (~55 GB/s/ring), so transfers are split finely: chunk 0 arrives as
    8 column-segments (PE starts after the first ~515-col segment);
    chunks 1-3 are partition-split 4/3/2-way to land before PE needs
    them. Issue cost is ~0.6us/dma_start per sequencer, so issues are
    spread over the SP and Scalar sequencers.
  - Output ships per 1024-col quarter as merges complete; the final
    quarter is partition-split 4-way to shorten the tail.
"""

import numpy as np

B, T, C, K = 8, 4096, 512, 4
P = 128  # partitions
NCHUNK = C // P  # 4 channel chunks
TJ = 512  # time-tile (free dim) per matmul; one PSUM bank
NJ = T // TJ  # 8 j-tiles per chunk
TP = T + K - 1  # padded time = 4099
TH = T // 2  # half-chunk = 2048 cols
TQ = T // 4  # quarter-chunk = 1024 cols

_compiled = None


def _build():
    import concourse.bacc as bacc
    import concourse.mybir as mybir
    from concourse.tile import TileContext

    f32 = mybir.dt.float32
    f16 = mybir.dt.float16
    nc = bacc.Bacc(enable_partition_id=False)

    wcol_d = nc.declare_dram_parameter("wt", [P, NCHUNK * K], f32, isOutput=False)
    ident_d = nc.declare_dram_parameter("ident", [P, P], f16, isOutput=False)
    xw_d = nc.declare_dram_parameter("xw", [P, NCHUNK * TP], f16, isOutput=False)
    out_d = nc.declare_dram_parameter("out", [C, T], f16, isOutput=True)

    with TileContext(nc) as tc:
        with (
            tc.tile_pool(name="xpool", bufs=1) as xpool,
            tc.tile_pool(name="wpool", bufs=1) as wpool,
            tc.tile_pool(name="opool", bufs=4) as opool,
            tc.tile_pool(name="ppool", bufs=2, space="PSUM") as ppool,
        ):
            wcol = wpool.tile([P, NCHUNK * K], f32, tag="wcol")
            ident = wpool.tile([P, P], f16, tag="ident")
            xts = [
                xpool.tile([P, TP], f16, name=f"xt{c}", tag=f"xt{c}")
                for c in range(NCHUNK)
            ]

            # --- SP issues: wcol, ident, then chunk 0 as 8 column segments
            nc.sync.dma_start(out=wcol, in_=wcol_d[:, :])
            nc.sync.dma_start(out=ident, in_=ident_d[:, :])
            seg = [0, TJ + K - 1]  # first segment covers j-tile 0 + halo
            while seg[-1] < TP:
                seg.append(min(seg[-1] + TJ, TP))
            for lo, hi in zip(seg[:-1], seg[1:]):
                nc.sync.dma_start(out=xts[0][:, lo:hi], in_=xw_d[:, lo:hi])

            # --- Scalar engine issues: chunks 1-3, partition-split so each
            # lands (on parallel rings) before PE finishes the prior chunk
            for c, nsplit in ((1, 4), (2, 3), (3, 2)):
                bounds = [P * i // nsplit for i in range(nsplit + 1)]
                for plo, phi in zip(bounds[:-1], bounds[1:]):
                    nc.scalar.dma_start(
                        out=xts[c][plo:phi, :],
                        in_=xw_d[plo:phi, c * TP : (c + 1) * TP],
                    )

            # --- Scalar engine builds diag stationaries for PE taps 0..2:
            # wt_c[:, k*P:(k+1)*P] = ident * w[:, c*K+k] (per-partition scale)
            wts = []
            for c in range(NCHUNK):
                wt = wpool.tile([P, 3 * P], f16, name=f"wd{c}", tag=f"wd{c}")
                wts.append(wt)
            for c in range(NCHUNK):
                for k in range(3):
                    idx = c * K + k
                    nc.scalar.mul(
                        wts[c][:, k * P : (k + 1) * P],
                        ident,
                        wcol[:, idx : idx + 1],
                    )

            # --- main loop: PE 3 taps -> PSUM; DVE fuses tap3 + drain
            for c in range(NCHUNK):
                xv = xts[c]
                wt = wts[c]
                w3 = wcol[:, c * K + 3 : c * K + 4]
                ot = opool.tile([P, T], f16, tag="ot")
                for half in range(2):
                    pt = ppool.tile([P, TH], f32, name="pt", tag="pt")
                    for j4 in range(NJ // 2):
                        j = half * (NJ // 2) + j4
                        for k in range(3):
                            nc.tensor.matmul(
                                pt[:, j4 * TJ : (j4 + 1) * TJ],
                                wt[:, k * P : (k + 1) * P],
                                xv[:, j * TJ + k : j * TJ + k + TJ],
                                start=(k == 0),
                                stop=(k == 2),
                            )
                    hbase = half * TH
                    last_chunk = c == NCHUNK - 1
                    nmerge = 2 if last_chunk else 1  # quarter-merges at tail
                    for m in range(nmerge):
                        mlo = hbase + m * (TH // nmerge)
                        mhi = mlo + TH // nmerge
                        nc.vector.scalar_tensor_tensor(
                            out=ot[:, mlo:mhi],
                            in0=xv[:, mlo + 3 : mhi + 3],
                            scalar=w3,
                            in1=pt[:, mlo - hbase : mhi - hbase],
                            op0=mybir.AluOpType.mult,
                            op1=mybir.AluOpType.add,
                        )
                    # ship output per quarter; SP covers chunks 0-1,
                    # Scalar chunks 2-3 (both sequencers stay busy)
                    eng = nc.sync if c < 2 else nc.scalar
                    for q in range(2):
                        qlo = hbase + q * TQ
                        qhi = qlo + TQ
                        if last_chunk and half == 1 and q == 1:
                            # final quarter: partition-split 4-way to
                            # shorten the tail on parallel rings
                            for pp in range(4):
                                plo, phi = 32 * pp, 32 * (pp + 1)
                                eng.dma_start(
                                    out=out_d[c * P + plo : c * P + phi, qlo:qhi],
                                    in_=ot[plo:phi, qlo:qhi],
                                )
                        else:
                            eng.dma_start(
                                out=out_d[c * P : (c + 1) * P, qlo:qhi],
                                in_=ot[:, qlo:qhi],
                            )

    nc.compile()
    return nc


def _prep_inputs(x: np.ndarray, weight: np.ndarray):
    # wcol[p, chunk*K + k] = weight[chunk*P + p, k]
    wcol = np.ascontiguousarray(
        weight.reshape(NCHUNK, P, K).transpose(1, 0, 2).reshape(P, NCHUNK * K)
    ).astype(np.float32)
    ident = np.eye(P, dtype=np.float16)
    xs = []
    for b in range(B):
        xp = np.zeros((C, TP), dtype=np.float32)
        xp[:, K - 1 :] = x[b].T  # [512, 4099], 3 leading zeros
        xw = np.ascontiguousarray(
            xp.reshape(NCHUNK, P, TP).transpose(1, 0, 2).reshape(P, NCHUNK * TP)
        ).astype(np.float16)
        xs.append(xw)
    return xs, wcol, ident


def _in_maps(x: np.ndarray, weight: np.ndarray):
    xs, wcol, ident = _prep_inputs(x, weight)
    return [{"xw": xs[b], "wt": wcol, "ident": ident} for b in range(B)]


def _ensure_axon_hooks():
    """This image's antenv package lacks axon_hooks; synthesize it so a
    trace=True / BASS_TRACE run of run_bass_kernel_spmd can profile
    instead of crashing on import."""
    import sys
    import types

    if "antenv.axon_hooks" in sys.modules:
        return
    mod = types.ModuleType("antenv.axon_hooks")
    state = {"hook": None}
    mod.set_axon_ntff_profile_hook = lambda h: state.__setitem__("hook", h)
    mod.get_axon_ntff_profile_hook = lambda: state["hook"]
    sys.modules["antenv.axon_hooks"] = mod
    try:
        if "/root/.axon_site" not in sys.path:
            sys.path.insert(0, "/root/.axon_site")
        from trn_agent_boot.trn_boot import _ntff_profile_via_ctypes

        mod.set_axon_ntff_profile_hook(
            _ntff_profile_via_ctypes("/opt/axon/libaxon_pjrt.so")
        )
    except Exception:
        pass  # hook stays None; concourse degrades to no-trace


def kernel(x: np.ndarray, weight: np.ndarray) -> np.ndarray:
    global _compiled
    _ensure_axon_hooks()
    from concourse import bass_utils

    x = np.ascontiguousarray(x, dtype=np.float32)
    weight = np.ascontiguousarray(weight, dtype=np.float32)

    if _compiled is None:
        _compiled = _build()
    nc = _compiled

    in_maps = _in_maps(x, weight)
    res = bass_utils.run_bass_kernel_spmd(nc, in_maps, core_ids=list(range(B)))

    out = np.empty((B, T, C), dtype=np.float32)
    for b in range(B):
        out[b] = np.asarray(res.results[b]["out"]).astype(np.float32).T
    return out


# revision 4
# speedup vs baseline: 1.7165x; 1.7165x over previous
"""Depthwise causal conv1d kernel for Trainium2 (8 NeuronCores, SPMD).

Problem: x [B=8, T=4096, C=512] f32, weight [C=512, K=4] f32.
out[b, t, c] = sum_k weight[c, k] * x[b, t - 3 + k, c]   (causal, zero-pad)

Strategy (v3):
  - Data-parallel over batch: core b handles x[b].
  - Host-side layout: channels-first x[b].T padded with 3 leading zeros
    along time -> [C=512, T+3=4099] fp16, reshaped to [128, 4*4099]
    (4 channel chunks of 128 on partitions). fp16 halves HBM traffic;
    accumulation stays fp32 in PSUM.
  - PE computes taps 0..2 as accumulating diag-matmuls (96 x 512-col
    matmuls, ~216ns each at full clock); DVE fuses tap 3 + PSUM drain +
    fp16 cast in ONE scalar_tensor_tensor per half-chunk:
    out = (x_shift3 * w3) + psum. Cuts PE time 25% vs the 4-tap
    baseline and eliminates the separate PSUM->SBUF copy pass.
  - Engine/sequencer roles are strictly separated (a sequencer that
    issues DMAs can't also feed compute without stalling it):
      SP-seq: arms the 6 input DMAs (wcol, identity, 4 x-chunks).
        DMA descriptors spray round-robin over all 16 rings, so one
        dma_start per chunk already lands in ~1.2us; no splitting.
      DVE: builds the 12 diag stationaries from a host-sent fp16
        identity via 4x-mode tensor_scalar (~150ns each), then merges.
      ACT-seq: arms all output DMAs up front; each waits in the rings
        on its merge semaphore and bursts (~0.6us) when it fires.
      PE: nothing but the 96 matmuls.
  - Output ships per half-chunk; the last chunk merges/ships per
    quarter to shorten the tail.
"""

import numpy as np

B, T, C, K = 8, 4096, 512, 4
P = 128  # partitions
NCHUNK = C // P  # 4 channel chunks
TJ = 512  # time-tile (free dim) per matmul; one PSUM bank
NJ = T // TJ  # 8 j-tiles per chunk
TP = T + K - 1  # padded time = 4099
TH = T // 2  # half-chunk = 2048 cols
TQ = T // 4  # quarter-chunk = 1024 cols

_compiled = None


def _build():
    import concourse.bacc as bacc
    import concourse.mybir as mybir
    from concourse.tile import TileContext

    f32 = mybir.dt.float32
    f16 = mybir.dt.float16
    nc = bacc.Bacc(enable_partition_id=False)

    wcol_d = nc.declare_dram_parameter("wt", [P, NCHUNK * K], f32, isOutput=False)
    ident_d = nc.declare_dram_parameter("ident", [P, P], f16, isOutput=False)
    xw_d = nc.declare_dram_parameter("xw", [P, NCHUNK * TP], f16, isOutput=False)
    out_d = nc.declare_dram_parameter("out", [C, T], f16, isOutput=True)

    with TileContext(nc) as tc:
        with (
            tc.tile_pool(name="xpool", bufs=1) as xpool,
            tc.tile_pool(name="wpool", bufs=1) as wpool,
            tc.tile_pool(name="opool", bufs=4) as opool,
            tc.tile_pool(name="ppool", bufs=2, space="PSUM") as ppool,
        ):
            wcol = wpool.tile([P, NCHUNK * K], f32, tag="wcol")
            ident = wpool.tile([P, P], f16, tag="ident")
            xts = [
                xpool.tile([P, TP], f16, name=f"xt{c}", tag=f"xt{c}")
                for c in range(NCHUNK)
            ]

            # --- SP arms all inputs; descriptors spray over 16 rings so
            # chunk c is fully resident ~1.2us after its arm processes
            nc.sync.dma_start(out=wcol, in_=wcol_d[:, :])
            nc.sync.dma_start(out=ident, in_=ident_d[:, :])
            for c in range(NCHUNK):
                nc.sync.dma_start(
                    out=xts[c], in_=xw_d[:, c * TP : (c + 1) * TP]
                )

            # --- DVE builds diag stationaries for PE taps 0..2 (4x-mode
            # tensor_scalar: wt_c[:, kP:(k+1)P] = ident * w[:, c*K+k])
            wts = []
            for c in range(NCHUNK):
                wt = wpool.tile([P, 3 * P], f16, name=f"wd{c}", tag=f"wd{c}")
                wts.append(wt)
            for c in range(NCHUNK):
                for k in range(3):
                    idx = c * K + k
                    nc.vector.tensor_scalar_mul(
                        wts[c][:, k * P : (k + 1) * P],
                        ident,
                        wcol[:, idx : idx + 1],
                    )

            # --- main loop: PE 3 taps -> PSUM; DVE fuses tap3 + drain
            for c in range(NCHUNK):
                xv = xts[c]
                wt = wts[c]
                w3 = wcol[:, c * K + 3 : c * K + 4]
                ot = opool.tile([P, T], f16, tag="ot")
                last_chunk = c == NCHUNK - 1
                for half in range(2):
                    pt = ppool.tile([P, TH], f32, name="pt", tag="pt")
                    for j4 in range(NJ // 2):
                        j = half * (NJ // 2) + j4
                        for k in range(3):
                            nc.tensor.matmul(
                                pt[:, j4 * TJ : (j4 + 1) * TJ],
                                wt[:, k * P : (k + 1) * P],
                                xv[:, j * TJ + k : j * TJ + k + TJ],
                                start=(k == 0),
                                stop=(k == 2),
                            )
                    hbase = half * TH
                    nmerge = 2 if last_chunk else 1  # quarter-merges at tail
                    for m in range(nmerge):
                        mlo = hbase + m * (TH // nmerge)
                        mhi = mlo + TH // nmerge
                        nc.vector.scalar_tensor_tensor(
                            out=ot[:, mlo:mhi],
                            in0=xv[:, mlo + 3 : mhi + 3],
                            scalar=w3,
                            in1=pt[:, mlo - hbase : mhi - hbase],
                            op0=mybir.AluOpType.mult,
                            op1=mybir.AluOpType.add,
                        )
                    # ACT arms the output DMA; its descriptors wait on the
                    # merge semaphore in the rings and burst when it fires
                    if last_chunk:
                        for q in range(2):
                            qlo = hbase + q * TQ
                            nc.scalar.dma_start(
                                out=out_d[c * P : (c + 1) * P, qlo : qlo + TQ],
                                in_=ot[:, qlo : qlo + TQ],
                            )
                    else:
                        nc.scalar.dma_start(
                            out=out_d[c * P : (c + 1) * P, hbase : hbase + TH],
                            in_=ot[:, hbase : hbase + TH],
                        )

    nc.compile()
    return nc


def _prep_inputs(x: np.ndarray, weight: np.ndarray):
    # wcol[p, chunk*K + k] = weight[chunk*P + p, k]
    wcol = np.ascontiguousarray(
        weight.reshape(NCHUNK, P, K).transpose(1, 0, 2).reshape(P, NCHUNK * K)
    ).astype(np.float32)
    ident = np.eye(P, dtype=np.float16)
    xs = []
    for b in range(B):
        xp = np.zeros((C, TP), dtype=np.float32)
        xp[:, K - 1 :] = x[b].T  # [512, 4099], 3 leading zeros
        xw = np.ascontiguousarray(
            xp.reshape(NCHUNK, P, TP).transpose(1, 0, 2).reshape(P, NCHUNK * TP)
        ).astype(np.float16)
        xs.append(xw)
    return xs, wcol, ident


def _in_maps(x: np.ndarray, weight: np.ndarray):
    xs, wcol, ident = _prep_inputs(x, weight)
    return [{"xw": xs[b], "wt": wcol, "ident": ident} for b in range(B)]


def _ensure_axon_hooks():
    """This image's antenv package lacks axon_hooks; synthesize it so a
    trace=True / BASS_TRACE run of run_bass_kernel_spmd can profile
    instead of crashing on import."""
    import sys
    import types

    if "antenv.axon_hooks" in sys.modules:
        return
    mod = types.ModuleType("antenv.axon_hooks")
    state = {"hook": None}
    mod.set_axon_ntff_profile_hook = lambda h: state.__setitem__("hook", h)
    mod.get_axon_ntff_profile_hook = lambda: state["hook"]
    sys.modules["antenv.axon_hooks"] = mod
    try:
        if "/root/.axon_site" not in sys.path:
            sys.path.insert(0, "/root/.axon_site")
        from trn_agent_boot.trn_boot import _ntff_profile_via_ctypes

        mod.set_axon_ntff_profile_hook(
            _ntff_profile_via_ctypes("/opt/axon/libaxon_pjrt.so")
        )
    except Exception:
        pass  # hook stays None; concourse degrades to no-trace


def kernel(x: np.ndarray, weight: np.ndarray) -> np.ndarray:
    global _compiled
    _ensure_axon_hooks()
    from concourse import bass_utils

    x = np.ascontiguousarray(x, dtype=np.float32)
    weight = np.ascontiguousarray(weight, dtype=np.float32)

    if _compiled is None:
        _compiled = _build()
    nc = _compiled

    in_maps = _in_maps(x, weight)
    res = bass_utils.run_bass_kernel_spmd(nc, in_maps, core_ids=list(range(B)))

    out = np.empty((B, T, C), dtype=np.float32)
    for b in range(B):
        out[b] = np.asarray(res.results[b]["out"]).astype(np.float32).T
    return out
